# revision 18
# baseline (speedup 1.0000x reference)
"""EMA (ExponentialMovingAverage, adjust=True) over (32, 4096, 256) f32 on 8 trn2 cores.

Math: the reference recurrence is
    e_0 = x_0;  e_t = (alpha*x_t + oma*e_{t-1}) / w_t,  w_t = max(1-oma^(t+1), 1e-10)
i.e. e_t = a_t*e_{t-1} + b_t*x_t with a_t = oma/w_t, b_t = alpha/w_t.

Chunk time into blocks of C=128. Within a chunk the scan is a lower-triangular
matmul E = W_k @ X (W_k[j,i] = b_{kC+i} * prod_{r=kC+i+1}^{kC+j} a_r). The carry
h = e_{kC-1} enters every row j with weight A_k[j] = (a_t/b_t)*W_k[j,0], and
a_t/b_t == oma/alpha for every t (w cancels), so the carry folds exactly into
the chunk's first input row: X'[0] = X[0] + (oma/alpha)*h. w_t == 1.0f for
t >= 216, so only chunks 0 and 1 have distinct W; chunks 2..31 share one W.

Sharding: pure data parallelism — 4 of the 32 batches per core, no comms.
Per (chunk, batch): one 128KB linear DMA in, one fp32 matmul (128x128 @
128x256), a PSUM->SBUF evict on the scalar engine, one 128KB linear DMA out;
the carry fold is a single (1,256) DVE op reading the previous chunk's PSUM
row 127.
"""

import os
import sys

import numpy as np

for _p in ("/opt/trn_rl_repo",):
    if os.path.isdir(_p) and _p not in sys.path:
        sys.path.append(_p)

import concourse.bass as bass
import concourse.mybir as mybir
from concourse.bass_utils import run_bass_kernel_spmd
from concourse.tile import TileContext
from concourse.vector_clock import ScopedClock

# ---------------------------------------------------------------------------
# Workaround: TileContext's tail drain puts every owed proc's sem wait on one
# Drain instruction; walrus codegen allows only one sync wait per instruction,
# so any kernel touching more than a few procs fails codegen with "Too many
# sync wait commands". Split the waits across SP nops, one wait each.
# ---------------------------------------------------------------------------
_MAX_WAITS = 1


def _split_drain_and_barrier(self, tick_clock, wait_clock):
    carrier = self.nc.sync.nop(nofuse=True, hint="drain_wait_carrier")
    wait_clock.add_sem_waits(
        carrier.ins, ScopedClock({None: tick_clock.global_clock})
    )
    si = carrier.ins.sync_info
    if si is not None and len(si.on_wait) > _MAX_WAITS:
        waits = list(si.on_wait)
        carrier.ins.sync_info = mybir.SyncInfo(
            on_wait=waits[:_MAX_WAITS], on_update=list(si.on_update)
        )
        rest = waits[_MAX_WAITS:]
        for i in range(0, len(rest), _MAX_WAITS):
            nop = self.nc.sync.nop(nofuse=True, hint="drain_wait_spill")
            nop.ins.sync_info = mybir.SyncInfo(
                on_wait=rest[i : i + _MAX_WAITS], on_update=[]
            )
    self.nc.sync.drain()

    self.nc.all_engine_barrier()
    assert self.sems is not None
    popped = self.nc._tile_sem_poison_stack.pop()
    assert popped is self._sem_poison
    self.nc.clear_and_free_semaphores(list(self.sems.allocated().values()))
    self.nc.all_engine_barrier()


TileContext._drain_and_barrier = _split_drain_and_barrier

# ---------------------------------------------------------------------------
# Same walrus limitation for regular instructions: Tile attaches up to ~4 sem
# waits to one instruction; this walrus rejects more than WAIT_CAPS[type] sync
# wait commands per instruction. Spill the extras onto same-engine NoOps
# inserted right before the instruction (engines execute their stream in BB
# order, so the waits still complete before the instruction runs).
# ---------------------------------------------------------------------------

_WAIT_CAP_DEFAULT = 1
_WAIT_CAPS = {
    "InstEventSemaphore": 2,
}
_spill_counter = [0]


def spill_excess_waits(nc):
    for fn in nc.m.functions:
        for bb in fn.blocks:
            insts = bb.instructions
            i = 0
            while i < len(insts):
                inst = insts[i]
                si = inst.sync_info
                if si is None or not si.on_wait:
                    i += 1
                    continue
                cap = _WAIT_CAPS.get(type(inst).__name__, _WAIT_CAP_DEFAULT)
                waits = list(si.on_wait)
                if len(waits) <= cap:
                    i += 1
                    continue
                keep = waits[-cap:]
                rest = waits[:-cap]
                inst.sync_info = mybir.SyncInfo(
                    on_wait=keep, on_update=list(si.on_update)
                )
                carriers = []
                for j in range(0, len(rest), _WAIT_CAP_DEFAULT):
                    _spill_counter[0] += 1
                    nop = mybir.InstNoOp(name=f"spillw-{_spill_counter[0]}")
                    nop.engine = inst.engine
                    nop.sync_info = mybir.SyncInfo(
                        on_wait=rest[j : j + _WAIT_CAP_DEFAULT], on_update=[]
                    )
                    carriers.append(nop)
                for off, nop in enumerate(carriers):
                    insts.insert(i + off, nop)
                i += len(carriers) + 1

B, T, F = 32, 4096, 256
NCORES = 8
BL = B // NCORES  # local batches per core
C = 128  # time chunk
NCHUNK = T // C


def _coeffs():
    alpha32 = np.float32(2.0 / 26.0)
    oma32 = np.float32(1.0 - 2.0 / 26.0)
    t = np.arange(1, T, dtype=np.float32)
    w32 = np.maximum(
        np.float32(1.0) - oma32 ** (t + np.float32(1.0)), np.float32(1e-10)
    ).astype(np.float32)
    a = np.zeros(T, dtype=np.float64)
    b = np.zeros(T, dtype=np.float64)
    a[1:] = np.float64(oma32) / w32.astype(np.float64)
    b[1:] = np.float64(alpha32) / w32.astype(np.float64)
    b[0] = 1.0

    def build_w(k):
        lo = k * C
        av = a[lo : lo + C]
        bv = b[lo : lo + C]
        g = np.ones(C, dtype=np.float64)
        for j in range(1, C):
            g[j] = g[j - 1] * av[j]
        return np.tril((g[:, None] / g[None, :]) * bv[None, :])

    w0, w1, wc = build_w(0), build_w(1), build_w(2)
    cfold = float(np.float64(oma32) / np.float64(alpha32))
    # Rotate output rows so the chunk's last timestep (the carry row) lands on
    # PSUM partition 96: engine APs must start on a quad boundary (0/32/64/96),
    # so partition 127 would be unreadable by the DVE carry-fold op. Partition
    # p holds t = (p+31) % 128: p in [0,96] -> t = p+31, p in [97,127] ->
    # t = p-97, so the store splits into two contiguous pieces.
    perm = (np.arange(C) + 31) % C
    w0, w1, wc = w0[perm], w1[perm], wc[perm]
    # lhsT layout per matrix: [t_in (partition), t_out]; stack -> (128, 3, 128)
    wt = np.stack([w0.T, w1.T, wc.T], axis=0).astype(np.float32)
    wt = np.ascontiguousarray(wt.transpose(1, 0, 2))
    return wt, cfold


_WT, _CFOLD = _coeffs()


def build_nc(repeats=1, variant="full", xbufs=6, ebufs=6, spill=True,
             bench_io=False):
    f32 = mybir.dt.float32
    nc = bass.Bass(trn_type="TRN2")
    if bench_io:
        # Timing-only NEFF: tiny external I/O (dispatch payload over axon is
        # per-call, ~100ms for the real 384MB), real traffic hits internal
        # DRAM scratch instead. Data is garbage; timing is identical.
        xin = nc.dram_tensor("x", [1, 4], f32, kind="ExternalInput")
        wt = nc.dram_tensor("wt", [128, 3, C], f32, kind="ExternalInput")
        yout = nc.dram_tensor("y", [1, 4], f32, kind="ExternalOutput")
        x = nc.dram_tensor("xscratch", [BL, T, F], f32)
        y = nc.dram_tensor("yscratch", [BL, T, F], f32)
    else:
        x = nc.dram_tensor("x", [BL, T, F], f32, kind="ExternalInput")
        wt = nc.dram_tensor("wt", [128, 3, C], f32, kind="ExternalInput")
        y = nc.dram_tensor("y", [BL, T, F], f32, kind="ExternalOutput")

    with TileContext(nc) as tc:
        with (
            tc.tile_pool(name="wpool", bufs=1) as wpool,
            tc.tile_pool(name="xpool", bufs=xbufs) as xpool,
            tc.tile_pool(name="epool", bufs=ebufs) as epool,
            tc.tile_pool(name="psum", bufs=8, space="PSUM") as ppool,
        ):
            w_tile = wpool.tile([128, 3, C], f32)
            nc.sync.dma_start(out=w_tile[:], in_=wt[:])
            if bench_io:
                iot = wpool.tile([1, 4], f32, name="iot")
                nc.sync.dma_start(out=iot[:], in_=xin[:])
                nc.sync.dma_start(out=yout[:], in_=iot[:])
            for _rep in range(repeats):
                _emit_pass(nc, tc, x, y, w_tile, xpool, epool, ppool, f32, variant)
    if spill:
        spill_excess_waits(nc)
    return nc


GROUP = 8  # chunks per DMA group (1 MB loads)


def _emit_pass(nc, tc, x, y, w_tile, xpool, epool, ppool, f32, variant="full"):
    if variant.startswith("dmabig"):
        # pure-DMA bandwidth probe with NCH chunks per DMA
        nch = int(variant[len("dmabig"):])
        xr = x.rearrange("b (g t) f -> b t g f", t=C)
        yr = y.rearrange("b (g t) f -> b t g f", t=C)
        for g0 in range(0, NCHUNK, nch):
            for b in range(BL):
                xt = xpool.tile([C, nch, F], f32, tag="xtb")
                nc.sync.dma_start(out=xt[:], in_=xr[b, :, g0 : g0 + nch, :])
                nc.scalar.dma_start(out=yr[b, :, g0 : g0 + nch, :], in_=xt[:])
        return
    do_fuse = variant == "full"
    do_mm = variant in ("full", "nofuse", "mmonly")
    do_evict = variant in ("full", "nofuse", "actcopy")
    xr = x.rearrange("b (g t) f -> b t g f", t=C)
    yr = y.rearrange("b (g t) f -> b t g f", t=C)
    prev_ps = [None] * BL
    for g0 in range(0, NCHUNK, GROUP):
        xts, ets = [], []
        for b in range(BL):
            xt = xpool.tile([C, GROUP, F], f32, tag="xt")
            nc.sync.dma_start(out=xt[:], in_=xr[b, :, g0 : g0 + GROUP, :])
            xts.append(xt)
            ets.append(
                epool.tile([C, GROUP, F], f32, tag="et", name=f"et_{g0}_{b}")
            )
        for j in range(GROUP):
            k = g0 + j
            wsel = 0 if k == 0 else (1 if k == 1 else 2)
            for b in range(BL):
                if do_fuse and k > 0:
                    # X'[0] = X[0] + (oma/alpha) * e_{kC-1}; the carry row
                    # sits at partition 96 of the previous chunk's PSUM tile
                    nc.vector.scalar_tensor_tensor(
                        out=xts[b][0:1, j, :],
                        in0=prev_ps[b][96:97, :],
                        scalar=_CFOLD,
                        in1=xts[b][0:1, j, :],
                        op0=mybir.AluOpType.mult,
                        op1=mybir.AluOpType.add,
                    )
                if do_mm:
                    pt = ppool.tile([C, F], f32, tag="pt")
                    nc.tensor.matmul(
                        pt[:], w_tile[:, wsel, :], xts[b][:, j, :],
                        start=True, stop=True,
                    )
                    prev_ps[b] = pt
                    if do_evict:
                        nc.scalar.copy(out=ets[b][:, j, :], in_=pt[:])
                elif do_evict:
                    nc.scalar.copy(out=ets[b][:, j, :], in_=xts[b][:, j, :])
        for b in range(BL):
            et = xts[b] if not do_evict else ets[b]
            # partition p holds t = (p+31)%128 of each chunk: two contiguous
            # store pieces. The big piece goes on the ACT ring, the small one
            # alternates SP/ACT to balance ring time.
            nc.scalar.dma_start(
                out=yr[b, 31:C, g0 : g0 + GROUP, :], in_=et[0:97, :, :]
            )
            eng = nc.sync if b % 2 == 0 else nc.scalar
            eng.dma_start(
                out=yr[b, 0:31, g0 : g0 + GROUP, :], in_=et[97:C, :, :]
            )


_NC = None


def get_nc():
    global _NC
    if _NC is None:
        _NC = build_nc()
    return _NC


def kernel(x):
    x = np.ascontiguousarray(np.asarray(x, dtype=np.float32))
    assert x.shape == (B, T, F), x.shape
    nc = get_nc()
    in_maps = [
        {"x": x[c * BL : (c + 1) * BL], "wt": _WT} for c in range(NCORES)
    ]
    res = run_bass_kernel_spmd(nc, in_maps, core_ids=list(range(NCORES)))
    return np.concatenate([res.results[c]["y"] for c in range(NCORES)], axis=0)
